# revision 14
# baseline (speedup 1.0000x reference)
"""Trainium2 Bass kernel for nn_AttentionD8 (dense transformer attention, D8 irreps).

Sharding: data-parallel over batch B=8 -> 8 NeuronCores, one batch element per
core. No collectives. Each core runs the full per-batch attention block.

Device layout: channel-major ("transposed") everywhere; the host pre-transposes
inputs/weights with numpy and post-transposes outputs, so the device never
transposes anything:
  - qkv projections compute y.T = Wq @ x.T directly.
  - attention computes S.T (keys j on partitions, queries i on free axis);
    exp(S.T) is exactly the stationary operand of the A@V matmul, and an extra
    ones-column appended to V yields the softmax denominators for free.
  - normalization happens after attention: reciprocal of the denominators,
    PE-broadcast to (96 x N) tiles via 0/1 selector matmuls, one elementwise
    multiply per assembled output tile.
  - biases enter via a ones-row appended to x.T and a bias-row on the weight
    (bf16), except bp_A1 which is added in fp32 on the PSUM eviction.

Within-head channel orders are chosen so every partition-interleaving
SBUF->SBUF assembly is ONE dma whose flat element stream matches on both
sides:
  - q/k packs: 1d rows d*4+bi, 2d rows 32+e*2+r (a contraction-order
    permutation, consistent between q and k, so S is unchanged).
  - V-pack columns use the same order, so the attention-output rows stream
    straight into the merged o-tiles.

The PE clock-gate (HAM) has hysteresis: the attention QK/AV interleave is
~99% busy with micro-holes and never *transitions* the gate, so whatever
state it enters with persists. Dense same-weight warmup bursts are issued at
kernel start and right before attention to enter warm (2.4 GHz).

Matmul operands are bf16; accumulation is fp32 in PSUM; softmax statistics
and normalization are fp32.
"""

import os
import sys

import numpy as np

for _p in ("/opt/trn_rl_repo", os.path.expanduser("~/.axon_site/_ro/trn_rl_repo")):
    if _p not in sys.path and os.path.isdir(_p):
        sys.path.append(_p)

import concourse.bass as bass  # noqa: F401
import concourse.tile as tile
from concourse import bacc, mybir
from concourse.bass_utils import run_bass_kernel_spmd

F32 = mybir.dt.float32
BF16 = mybir.dt.bfloat16
EXP = mybir.ActivationFunctionType.Exp

B, N, H, C = 8, 1024, 12, 96
HD = 64
NC2 = (slice(0, 512), slice(512, 1024))


def build():
    nc = bacc.Bacc("TRN2", target_bir_lowering=False, debug=False, num_devices=8)

    def inp(name, shape, dt=BF16):
        return nc.dram_tensor(name, list(shape), dt, kind="ExternalInput").ap()

    def outp(name, shape):
        return nc.dram_tensor(name, list(shape), F32, kind="ExternalOutput").ap()

    xa = [inp("xa1t", (97, N)), inp("xa2t", (96, N)), inp("xb1t", (96, N)),
          inp("xb2t", (96, N))]
    x2 = {(r, k): inp(f"x2t_{r}_{k}", (96, N)) for r in range(2) for k in range(2)}
    wq = [inp("wqa1t", (97, 288)), inp("wqa2t", (96, 288)), inp("wqb1t", (96, 288)),
          inp("wqb2t", (96, 288))]
    wqe = [inp("wqet_0", (96, 576)), inp("wqet_1", (96, 576))]
    wp = [inp("wpa1t", (96, 96)), inp("wpa2t", (96, 96)), inp("wpb1t", (96, 96)),
          inp("wpb2t", (96, 96))]
    wpe = [inp("wpet_0", (96, 192)), inp("wpet_1", (96, 192))]
    eb1d = inp("eb1", (12, 96), F32)
    ebed = inp("ebe", (12, 192), F32)
    bpa1d = inp("bpa1", (96, 1), F32)

    zd = [outp("z1t", (96, N)), outp("z2t", (96, N)), outp("z3t", (96, N)),
          outp("z4t", (96, N))]
    zed = [outp("zet_0", (192, N)), outp("zet_1", (192, N))]

    with tile.TileContext(nc) as tc, \
         tc.tile_pool(name="w", bufs=1) as wpool, \
         tc.tile_pool(name="pack", bufs=1) as packpool:
        # ---- constant / weight loads (wqe0 first: it feeds the warmup) ----
        wqes = [wpool.tile([96, 576], BF16, name=f"wqe{k}") for k in range(2)]
        wqs = [wpool.tile([t.shape[0], 288], BF16, name=f"wq{i}")
               for i, t in enumerate(wq)]
        wps = [wpool.tile([96, 96], BF16, name=f"wp{i}") for i in range(4)]
        wpes = [wpool.tile([96, 192], BF16, name=f"wpe{k}") for k in range(2)]
        for t, d in zip(wqes + wqs + wps + wpes, wqe + wq + wp + wpe):
            nc.sync.dma_start(t[:], d[:])
        eb1 = wpool.tile([12, 96], F32)
        nc.sync.dma_start(eb1[:], eb1d[:])
        ebe = wpool.tile([12, 192], F32)
        nc.sync.dma_start(ebe[:], ebed[:])
        bpa1 = wpool.tile([96, 1], F32)
        nc.sync.dma_start(bpa1[:], bpa1d[:])

        qpack = [packpool.tile([128, N], BF16, name=f"qp{i}") for i in range(6)]
        kpack = [packpool.tile([128, N], BF16, name=f"kp{i}") for i in range(6)]
        vpack = [packpool.tile([128, 12 * 65], BF16, name=f"vp{i}") for i in range(8)]

        with tc.tile_pool(name="x", bufs=1) as xpool, \
             tc.tile_pool(name="y", bufs=1) as ypool, \
             tc.tile_pool(name="pj", bufs=2, space="PSUM") as pj:

            xs = [xpool.tile([t.shape[0], N], BF16, name=f"x{i}")
                  for i, t in enumerate(xa)]
            for i, (t, d) in enumerate(zip(xs, xa)):
                [nc.sync, nc.gpsimd][i % 2].dma_start(t[:], d[:])
            x2s = {}
            for r in range(2):
                for k in range(2):
                    x2s[r, k] = xpool.tile([96, N], BF16, name=f"x2{r}{k}")
                    [nc.sync, nc.gpsimd][(r + k) % 2].dma_start(
                        x2s[r, k][:], x2[r, k][:])

            # ---- PE warmup. The HAM busy-detector only fires on a dense
            # same-weight matmul stream (LDWEIGHTS micro-holes defeat it), so
            # real phases can never warm themselves; once warm, the state
            # persists as long as no idle gap exceeds ~3.4us. Burn ~20 matmuls
            # on the first-loaded weight tile, then chain through the x tiles
            # (same stationary operand, so the stream stays dense) so the
            # burst cannot finish before phase A's inputs have landed. ----
            for i in range(20):
                wu = pj.tile([96, 512], F32, tag="pj", name="wu")
                nc.tensor.matmul(wu[:], wqes[0][:, :96], wqes[0][:, :512],
                                 start=True, stop=True)
            for i, rhs in enumerate(xs + list(x2s.values())):
                for ic in range(2):
                    wu = pj.tile([96, 512], F32, tag="pj", name="wu")
                    nc.tensor.matmul(wu[:], wqes[0][:96, :96],
                                     rhs[:96, NC2[ic]], start=True, stop=True)

            # y layout: 1d merged per qk -> (96 rows h*8+d) x (4 irreps x N);
            # 2d merged per (qk, head-half) -> (96 rows (h%6)*16+e) x (2 r x N)
            y1d = [ypool.tile([96, 4 * N], BF16, name=f"y1d{qk}") for qk in range(2)]
            y2d = {(qk, hh): ypool.tile([96, 2 * N], BF16, name=f"y2d{qk}{hh}")
                   for qk in range(2) for hh in range(2)}

            # ---- phase A: q/k projections ----
            for bi in range(4):
                for qk in range(2):
                    for ic in range(2):
                        ps = pj.tile([96, 512], F32, tag="pj", name="ps_pj")
                        nc.tensor.matmul(
                            ps[:],
                            wqs[bi][:, qk * 96:(qk + 1) * 96],
                            xs[bi][:, NC2[ic]],
                            start=True, stop=True,
                        )
                        nc.vector.tensor_copy(
                            y1d[qk][:, bi * N + ic * 512: bi * N + ic * 512 + 512],
                            ps[:])
            for r in range(2):
                for oc in range(4):  # oc = qk*2 + hh
                    qk, hh = divmod(oc, 2)
                    for ic in range(2):
                        ps = pj.tile([96, 512], F32, tag="pj", name="ps_pj")
                        for k in range(2):
                            nc.tensor.matmul(
                                ps[:],
                                wqes[k][:, oc * 96:(oc + 1) * 96],
                                x2s[r, k][:, NC2[ic]],
                                start=(k == 0), stop=(k == 1),
                            )
                        nc.vector.tensor_copy(
                            y2d[qk, hh][:, r * N + ic * 512: r * N + ic * 512 + 512],
                            ps[:])

            # ---- phase C: v projections, token-major, into packed v tiles ----
            # vpack column order per head: [1d: d*4+bi | 2d: 32+e*2+r | ones]
            for t8 in range(8):
                tok = slice(t8 * 128, (t8 + 1) * 128)
                v3 = vpack[t8].rearrange("p (h c) -> p h c", c=65)
                v1dv = v3[:, :, 0:32].rearrange("p h (d b) -> p h d b", b=4)
                v2dv = v3[:, :, 32:64].rearrange("p h (e r) -> p h e r", r=2)
                for bi in range(4):
                    ps = pj.tile([128, 96], F32, tag="pjv", name="ps_pjv")
                    nc.tensor.matmul(
                        ps[:], xs[bi][:, tok], wqs[bi][:, 192:288],
                        start=True, stop=True,
                    )
                    nc.vector.tensor_copy(
                        v1dv[:, :, :, bi],
                        ps.rearrange("p (h d) -> p h d", d=8),
                    )
                for r in range(2):
                    ps = pj.tile([128, 192], F32, tag="pjv", name="ps_pjv")
                    for k in range(2):
                        nc.tensor.matmul(
                            ps[:], x2s[r, k][:, tok], wqes[k][:, 384:576],
                            start=(k == 0), stop=(k == 1),
                        )
                    nc.vector.tensor_copy(
                        v2dv[:, :, :, r],
                        ps.rearrange("p (h e) -> p h e", e=16),
                    )
                nc.vector.memset(v3[:, :, 64:65], 1.0)

            # ---- phase B: assemble q/k packs. One dma per (head, qk, 1d/2d);
            # flat element streams match (dst is a plain 2D partition slice).
            # Issue alternates sync/gpsimd queues. ----
            for h in range(H):
                hp, p = divmod(h, 2)
                eng = [nc.sync, nc.gpsimd][h % 2]
                for qk, pack in ((0, qpack), (1, kpack)):
                    base = p * 64
                    eng.dma_start(
                        pack[hp][base: base + 32, :],
                        y1d[qk][h * 8:(h + 1) * 8, :].rearrange(
                            "d (b t) -> d b t", b=4),
                    )
                    eng.dma_start(
                        pack[hp][base + 32: base + 64, :],
                        y2d[qk, h // 6][(h % 6) * 16:(h % 6 + 1) * 16, :].rearrange(
                            "e (r t) -> e r t", r=2),
                    )

        # ---- phases D/E: attention, normalize, output projections ----
        with tc.tile_pool(name="o", bufs=1) as opool:
            # merged assembled outputs: o1d rows h*8+d, free (bi, t);
            # oe[kc] rows (h%6)*16+e, free (r, t)
            o1d = opool.tile([96, 4 * N], F32, name="o1d")
            oe = [opool.tile([96, 2 * N], F32, name=f"oe{kc}") for kc in range(2)]
            rin = opool.tile([12, N], F32)

            with tc.tile_pool(name="es", bufs=16) as espool, \
                 tc.tile_pool(name="avsp", bufs=3) as avspool, \
                 tc.tile_pool(name="st", bufs=2, space="PSUM") as stp, \
                 tc.tile_pool(name="av", bufs=2, space="PSUM") as avp:

                # bridge warmup: keep/restore HAM warm across the assembly gap
                for i in range(15):
                    wu = stp.tile([96, 512], F32, tag="st", name="wu2")
                    nc.tensor.matmul(wu[:], wqes[0][:, :96], wqes[0][:, :512],
                                     start=True, stop=True)

                def drain_head(h, avps):
                    avs = avspool.tile([65, N], F32, tag="avs", name="avs")
                    nc.vector.tensor_copy(avs[:], avps[:])
                    nc.gpsimd.dma_start(o1d[h * 8:(h + 1) * 8, :], avs[0:32, :])
                    nc.gpsimd.dma_start(
                        oe[h // 6][(h % 6) * 16:(h % 6 + 1) * 16, :], avs[32:64, :])
                    nc.gpsimd.dma_start(rin[h:h + 1, :], avs[64:65, :])

                es_prev, av_prev = None, None
                for h in range(H + 1):
                    hp, p = divmod(h, 2)
                    base = p * 64
                    es_cur = []
                    avps = (avp.tile([65, N], F32, tag="av", name="ps_av")
                            if h < H else None)
                    for jc in range(8):
                        if h < H:
                            stps = stp.tile([128, N], F32, tag="st", name="ps_st")
                            for ic in range(2):
                                nc.tensor.matmul(
                                    stps[:, NC2[ic]],
                                    kpack[hp][base:base + 64, jc * 128:(jc + 1) * 128],
                                    qpack[hp][base:base + 64, NC2[ic]],
                                    start=True, stop=True,
                                )
                            es = espool.tile([128, N], BF16, tag="es", name="es")
                            nc.scalar.activation(es[:], stps[:], EXP,
                                                 scale=float(HD) ** -0.5)
                            es_cur.append(es)
                        if h > 0:
                            for ic in range(2):
                                nc.tensor.matmul(
                                    av_prev[:, NC2[ic]],
                                    vpack[jc][:, (h - 1) * 65:h * 65],
                                    es_prev[jc][:, NC2[ic]],
                                    start=(jc == 0), stop=(jc == 7),
                                )
                    if h > 0:
                        drain_head(h - 1, av_prev)
                    es_prev, av_prev = es_cur, avps

                # fill the attention->epilogue dependency gap (waiting on the
                # denominator DMAs + reciprocal) with warm same-weight matmuls
                # so the normalize/z-projection matmuls run at 2.4 GHz
                for i in range(30):
                    wu = stp.tile([96, 512], F32, tag="st", name="wu3")
                    nc.tensor.matmul(wu[:], wqes[0][:, :96], wqes[0][:, :512],
                                     start=True, stop=True)

            # ---- phase E: normalize (fp32) + output projections (bf16) ----
            with tc.tile_pool(name="on", bufs=1) as onpool, \
                 tc.tile_pool(name="z", bufs=4) as zpool, \
                 tc.tile_pool(name="ep", bufs=2, space="PSUM") as epp:
                otn = [onpool.tile([96, N], BF16, name=f"on{i}") for i in range(4)]
                oetn = {(r, k): onpool.tile([96, N], BF16, name=f"oen{r}{k}")
                        for r in range(2) for k in range(2)}
                rec = opool.tile([12, N], F32)
                nc.vector.reciprocal_approx_fast(rec[:], rin[:])
                bc1 = epp.tile([96, N], F32, tag="bc", name="ps_bc")
                for ic in range(2):
                    nc.tensor.matmul(bc1[:, NC2[ic]], eb1[:], rec[:, NC2[ic]],
                                     start=True, stop=True)
                for bi in range(4):
                    nc.vector.tensor_mul(otn[bi][:], o1d[:, bi * N:(bi + 1) * N],
                                         bc1[:])
                for k in range(2):
                    bce = epp.tile([96, N], F32, tag="bc", name="ps_bc")
                    for ic in range(2):
                        nc.tensor.matmul(bce[:, NC2[ic]],
                                         ebe[:, k * 96:(k + 1) * 96],
                                         rec[:, NC2[ic]], start=True, stop=True)
                    for r in range(2):
                        nc.vector.tensor_mul(oetn[r, k][:],
                                             oe[k][:, r * N:(r + 1) * N], bce[:])

                for zi in range(4):
                    zs = zpool.tile([96, N], F32, tag="z", name="zs")
                    for ic in range(2):
                        ps = epp.tile([96, 512], F32, tag="zps", name="ps_z")
                        nc.tensor.matmul(ps[:], wps[zi][:], otn[zi][:, NC2[ic]],
                                         start=True, stop=True)
                        if zi == 0:
                            nc.vector.tensor_scalar_add(zs[:, NC2[ic]], ps[:],
                                                        bpa1[:])
                        else:
                            nc.vector.tensor_copy(zs[:, NC2[ic]], ps[:])
                    nc.sync.dma_start(zd[zi][:], zs[:])
                for r in range(2):
                    for mc in range(2):
                        zs = zpool.tile([96, N], F32, tag="z", name="zs")
                        for ic in range(2):
                            ps = epp.tile([96, 512], F32, tag="zps", name="ps_z")
                            for k in range(2):
                                nc.tensor.matmul(
                                    ps[:], wpes[k][:, mc * 96:(mc + 1) * 96],
                                    oetn[r, k][:, NC2[ic]],
                                    start=(k == 0), stop=(k == 1),
                                )
                            nc.vector.tensor_copy(zs[:, NC2[ic]], ps[:])
                        nc.sync.dma_start(zed[r][mc * 96:(mc + 1) * 96, :], zs[:])

    nc.compile()
    return nc


def make_in_maps(inputs):
    from ml_dtypes import bfloat16

    b16 = lambda a: np.ascontiguousarray(np.asarray(a, dtype=np.float32)).astype(bfloat16)  # noqa: E731
    f32c = lambda a: np.ascontiguousarray(a, dtype=np.float32)  # noqa: E731
    ones = np.ones((1, N), np.float32)
    shared = {
        "wqa1t": b16(np.concatenate(
            [np.asarray(inputs["wq_A1"], np.float32).T,
             np.asarray(inputs["bq_A1"], np.float32)[None, :]], 0)),
        "wqa2t": b16(np.asarray(inputs["wq_A2"]).T),
        "wqb1t": b16(np.asarray(inputs["wq_B1"]).T),
        "wqb2t": b16(np.asarray(inputs["wq_B2"]).T),
        "wpa1t": b16(np.asarray(inputs["wp_A1"]).T),
        "wpa2t": b16(np.asarray(inputs["wp_A2"]).T),
        "wpb1t": b16(np.asarray(inputs["wp_B1"]).T),
        "wpb2t": b16(np.asarray(inputs["wp_B2"]).T),
        "eb1": (np.arange(96)[None, :] // 8 == np.arange(12)[:, None]).astype(np.float32),
        "ebe": (np.arange(192)[None, :] // 16 == np.arange(12)[:, None]).astype(np.float32),
        "bpa1": f32c(np.asarray(inputs["bp_A1"], np.float32)[:, None]),
    }
    for k in range(2):
        shared[f"wqet_{k}"] = b16(np.asarray(inputs["wq_E"]).T[k * 96:(k + 1) * 96])
        shared[f"wpet_{k}"] = b16(np.asarray(inputs["wp_E"]).T[k * 96:(k + 1) * 96])
    maps = []
    for b in range(B):
        m = dict(shared)
        m["xa1t"] = b16(np.concatenate(
            [np.asarray(inputs["x_A1"][b], np.float32).T, ones], 0))
        m["xa2t"] = b16(np.asarray(inputs["x_A2"][b]).T)
        m["xb1t"] = b16(np.asarray(inputs["x_B1"][b]).T)
        m["xb2t"] = b16(np.asarray(inputs["x_B2"][b]).T)
        for r in range(2):
            for k in range(2):
                m[f"x2t_{r}_{k}"] = b16(
                    np.asarray(inputs["x_2d"][b, :, r, k * 96:(k + 1) * 96]).T)
        maps.append(m)
    return maps


def assemble_outputs(results):
    z = [np.empty((B, N, 96), np.float32) for _ in range(4)]
    ze = np.empty((B, N, 2, 192), np.float32)
    for b in range(B):
        for i in range(4):
            z[i][b] = results[b][f"z{i + 1}t"].T
        for r in range(2):
            ze[b, :, r, :] = results[b][f"zet_{r}"].T
    return z[0], z[1], z[2], z[3], ze


_NC_CACHE = {}


def kernel(**inputs):
    if "nc" not in _NC_CACHE:
        _NC_CACHE["nc"] = build()
    nc = _NC_CACHE["nc"]
    res = run_bass_kernel_spmd(nc, make_in_maps(inputs), list(range(B)))
    return assemble_outputs(res.results)


# revision 15
# speedup vs baseline: 1.3208x; 1.3208x over previous
"""Trainium2 Bass kernel for nn_AttentionD8 (dense transformer attention, D8 irreps).

Sharding: data-parallel over batch B=8 -> 8 NeuronCores, one batch element per
core. No collectives. Each core runs the full per-batch attention block.

Device layout: channel-major ("transposed") everywhere; the host pre-transposes
inputs/weights with numpy and post-transposes outputs, so the device never
transposes anything:
  - qkv projections compute y.T = Wq @ x.T directly.
  - attention computes S.T (keys j on partitions, queries i on free axis);
    exp(S.T) is exactly the stationary operand of the A@V matmul, and an extra
    ones-column appended to V yields the softmax denominators for free.
  - normalization happens after attention: reciprocal of the denominators,
    PE-broadcast to (96 x N) tiles via 0/1 selector matmuls, one elementwise
    multiply per assembled output tile.
  - biases enter via a ones-row appended to x.T and a bias-row on the weight
    (bf16), except bp_A1 which is added in fp32 on the PSUM eviction.

Within-head channel orders are chosen so every partition-interleaving
SBUF->SBUF assembly is ONE dma whose flat element stream matches on both
sides:
  - q/k packs: 1d rows d*4+bi, 2d rows 32+e*2+r (a contraction-order
    permutation, consistent between q and k, so S is unchanged).
  - V-pack columns use the same order, so the attention-output rows stream
    straight into the merged o-tiles.

The PE clock-gate (HAM) has hysteresis: the attention QK/AV interleave is
~99% busy with micro-holes and never *transitions* the gate, so whatever
state it enters with persists. Dense same-weight warmup bursts are issued at
kernel start and right before attention to enter warm (2.4 GHz).

Matmul operands are bf16; accumulation is fp32 in PSUM; softmax statistics
and normalization are fp32.
"""

import os
import sys

import numpy as np

for _p in ("/opt/trn_rl_repo", os.path.expanduser("~/.axon_site/_ro/trn_rl_repo")):
    if _p not in sys.path and os.path.isdir(_p):
        sys.path.append(_p)

import concourse.bass as bass  # noqa: F401
import concourse.tile as tile
from concourse import bacc, mybir
from concourse.bass_utils import run_bass_kernel_spmd

F32 = mybir.dt.float32
BF16 = mybir.dt.bfloat16
EXP = mybir.ActivationFunctionType.Exp

B, N, H, C = 8, 1024, 12, 96
HD = 64
NC2 = (slice(0, 512), slice(512, 1024))


def build():
    nc = bacc.Bacc("TRN2", target_bir_lowering=False, debug=False, num_devices=8)

    def inp(name, shape, dt=BF16):
        return nc.dram_tensor(name, list(shape), dt, kind="ExternalInput").ap()

    def outp(name, shape):
        return nc.dram_tensor(name, list(shape), F32, kind="ExternalOutput").ap()

    xa = [inp("xa1t", (97, N)), inp("xa2t", (96, N)), inp("xb1t", (96, N)),
          inp("xb2t", (96, N))]
    x2 = {(r, k): inp(f"x2t_{r}_{k}", (96, N)) for r in range(2) for k in range(2)}
    wq = [inp("wqa1t", (97, 288)), inp("wqa2t", (96, 288)), inp("wqb1t", (96, 288)),
          inp("wqb2t", (96, 288))]
    wqe = [inp("wqet_0", (96, 576)), inp("wqet_1", (96, 576))]
    wp = [inp("wpa1t", (96, 96)), inp("wpa2t", (96, 96)), inp("wpb1t", (96, 96)),
          inp("wpb2t", (96, 96))]
    wpe = [inp("wpet_0", (96, 192)), inp("wpet_1", (96, 192))]
    eb1d = inp("eb1", (12, 96), F32)
    ebed = inp("ebe", (12, 192), F32)
    bpa1d = inp("bpa1", (96, 1), F32)

    zd = [outp("z1t", (96, N)), outp("z2t", (96, N)), outp("z3t", (96, N)),
          outp("z4t", (96, N))]
    zed = [outp("zet_0", (192, N)), outp("zet_1", (192, N))]

    with tile.TileContext(nc) as tc, \
         tc.tile_pool(name="w", bufs=1) as wpool, \
         tc.tile_pool(name="pack", bufs=1) as packpool:
        # ---- constant / weight loads (wqe0 first: it feeds the warmup) ----
        wqes = [wpool.tile([96, 576], BF16, name=f"wqe{k}") for k in range(2)]
        wqs = [wpool.tile([t.shape[0], 288], BF16, name=f"wq{i}")
               for i, t in enumerate(wq)]
        wps = [wpool.tile([96, 96], BF16, name=f"wp{i}") for i in range(4)]
        wpes = [wpool.tile([96, 192], BF16, name=f"wpe{k}") for k in range(2)]
        for t, d in zip(wqes + wqs + wps + wpes, wqe + wq + wp + wpe):
            nc.sync.dma_start(t[:], d[:])
        eb1 = wpool.tile([12, 96], F32)
        nc.sync.dma_start(eb1[:], eb1d[:])
        ebe = wpool.tile([12, 192], F32)
        nc.sync.dma_start(ebe[:], ebed[:])
        bpa1 = wpool.tile([96, 1], F32)
        nc.sync.dma_start(bpa1[:], bpa1d[:])

        qpack = [packpool.tile([128, N], BF16, name=f"qp{i}") for i in range(6)]
        kpack = [packpool.tile([128, N], BF16, name=f"kp{i}") for i in range(6)]
        vpack = [packpool.tile([128, 12 * 65], BF16, name=f"vp{i}") for i in range(8)]

        with tc.tile_pool(name="x", bufs=1) as xpool, \
             tc.tile_pool(name="y", bufs=1) as ypool, \
             tc.tile_pool(name="pj", bufs=2, space="PSUM") as pj:

            xs = [xpool.tile([t.shape[0], N], BF16, name=f"x{i}")
                  for i, t in enumerate(xa)]
            for i, (t, d) in enumerate(zip(xs, xa)):
                [nc.sync, nc.gpsimd][i % 2].dma_start(t[:], d[:])
            x2s = {}
            for r in range(2):
                for k in range(2):
                    x2s[r, k] = xpool.tile([96, N], BF16, name=f"x2{r}{k}")
                    [nc.sync, nc.gpsimd][(r + k) % 2].dma_start(
                        x2s[r, k][:], x2[r, k][:])

            # ---- PE warmup. The HAM busy-detector only fires on a dense
            # same-weight matmul stream (LDWEIGHTS micro-holes defeat it), so
            # real phases can never warm themselves; once warm, the state
            # persists as long as no idle gap exceeds ~3.4us. Burn ~20 matmuls
            # on the first-loaded weight tile, then chain through the x tiles
            # (same stationary operand, so the stream stays dense) so the
            # burst cannot finish before phase A's inputs have landed. ----
            for i in range(20):
                wu = pj.tile([96, 512], F32, tag="pj", name="wu")
                nc.tensor.matmul(wu[:], wqes[0][:, :96], wqes[0][:, :512],
                                 start=True, stop=True)
            for i, rhs in enumerate(xs + list(x2s.values())):
                for ic in range(2):
                    wu = pj.tile([96, 512], F32, tag="pj", name="wu")
                    nc.tensor.matmul(wu[:], wqes[0][:96, :96],
                                     rhs[:96, NC2[ic]], start=True, stop=True)

            # y layout: 1d merged per qk -> (96 rows h*8+d) x (4 irreps x N);
            # 2d merged per (qk, head-half) -> (96 rows (h%6)*16+e) x (2 r x N)
            y1d = [ypool.tile([96, 4 * N], BF16, name=f"y1d{qk}") for qk in range(2)]
            y2d = {(qk, hh): ypool.tile([96, 2 * N], BF16, name=f"y2d{qk}{hh}")
                   for qk in range(2) for hh in range(2)}

            # ---- phase A: q/k projections ----
            for bi in range(4):
                for qk in range(2):
                    for ic in range(2):
                        ps = pj.tile([96, 512], F32, tag="pj", name="ps_pj")
                        nc.tensor.matmul(
                            ps[:],
                            wqs[bi][:, qk * 96:(qk + 1) * 96],
                            xs[bi][:, NC2[ic]],
                            start=True, stop=True,
                        )
                        nc.vector.tensor_copy(
                            y1d[qk][:, bi * N + ic * 512: bi * N + ic * 512 + 512],
                            ps[:])
            for r in range(2):
                for oc in range(4):  # oc = qk*2 + hh
                    qk, hh = divmod(oc, 2)
                    for ic in range(2):
                        ps = pj.tile([96, 512], F32, tag="pj", name="ps_pj")
                        for k in range(2):
                            nc.tensor.matmul(
                                ps[:],
                                wqes[k][:, oc * 96:(oc + 1) * 96],
                                x2s[r, k][:, NC2[ic]],
                                start=(k == 0), stop=(k == 1),
                            )
                        nc.vector.tensor_copy(
                            y2d[qk, hh][:, r * N + ic * 512: r * N + ic * 512 + 512],
                            ps[:])

            # ---- phase C: v projections, token-major, into packed v tiles ----
            # vpack column order per head: [1d: d*4+bi | 2d: 32+e*2+r | ones]
            for t8 in range(8):
                tok = slice(t8 * 128, (t8 + 1) * 128)
                v3 = vpack[t8].rearrange("p (h c) -> p h c", c=65)
                v1dv = v3[:, :, 0:32].rearrange("p h (d b) -> p h d b", b=4)
                v2dv = v3[:, :, 32:64].rearrange("p h (e r) -> p h e r", r=2)
                for bi in range(4):
                    ps = pj.tile([128, 96], F32, tag="pjv", name="ps_pjv")
                    nc.tensor.matmul(
                        ps[:], xs[bi][:, tok], wqs[bi][:, 192:288],
                        start=True, stop=True,
                    )
                    nc.vector.tensor_copy(
                        v1dv[:, :, :, bi],
                        ps.rearrange("p (h d) -> p h d", d=8),
                    )
                for r in range(2):
                    ps = pj.tile([128, 192], F32, tag="pjv", name="ps_pjv")
                    for k in range(2):
                        nc.tensor.matmul(
                            ps[:], x2s[r, k][:, tok], wqes[k][:, 384:576],
                            start=(k == 0), stop=(k == 1),
                        )
                    nc.vector.tensor_copy(
                        v2dv[:, :, :, r],
                        ps.rearrange("p (h e) -> p h e", e=16),
                    )
                nc.vector.memset(v3[:, :, 64:65], 1.0)

            # ---- phase B: assemble q/k packs. One dma per (head, qk, 1d/2d);
            # flat element streams match (dst is a plain 2D partition slice).
            # Issue alternates sync/gpsimd queues. ----
            for h in range(H):
                hp, p = divmod(h, 2)
                eng = [nc.sync, nc.gpsimd][h % 2]
                for qk, pack in ((0, qpack), (1, kpack)):
                    base = p * 64
                    eng.dma_start(
                        pack[hp][base: base + 32, :],
                        y1d[qk][h * 8:(h + 1) * 8, :].rearrange(
                            "d (b t) -> d b t", b=4),
                    )
                    eng.dma_start(
                        pack[hp][base + 32: base + 64, :],
                        y2d[qk, h // 6][(h % 6) * 16:(h % 6 + 1) * 16, :].rearrange(
                            "e (r t) -> e r t", r=2),
                    )

        # ---- phases D/E: attention, normalize, output projections ----
        with tc.tile_pool(name="o", bufs=1) as opool:
            # merged assembled outputs: o1d rows h*8+d, free (bi, t);
            # oe[kc] rows (h%6)*16+e, free (r, t)
            o1d = opool.tile([96, 4 * N], F32, name="o1d")
            oe = [opool.tile([96, 2 * N], F32, name=f"oe{kc}") for kc in range(2)]
            rin = opool.tile([12, N], F32)

            with tc.tile_pool(name="es", bufs=16) as espool, \
                 tc.tile_pool(name="avsp", bufs=3) as avspool, \
                 tc.tile_pool(name="st", bufs=2, space="PSUM") as stp, \
                 tc.tile_pool(name="av", bufs=2, space="PSUM") as avp:

                # bridge warmup: keep/restore HAM warm across the assembly gap
                for i in range(15):
                    wu = stp.tile([96, 512], F32, tag="st", name="wu2")
                    nc.tensor.matmul(wu[:], wqes[0][:, :96], wqes[0][:, :512],
                                     start=True, stop=True)

                def drain_head(h, avps):
                    avs = avspool.tile([65, N], F32, tag="avs", name="avs")
                    nc.vector.tensor_copy(avs[:], avps[:])
                    nc.gpsimd.dma_start(o1d[h * 8:(h + 1) * 8, :], avs[0:32, :])
                    nc.gpsimd.dma_start(
                        oe[h // 6][(h % 6) * 16:(h % 6 + 1) * 16, :], avs[32:64, :])
                    nc.gpsimd.dma_start(rin[h:h + 1, :], avs[64:65, :])

                es_prev, av_prev = None, None
                for h in range(H + 1):
                    hp, p = divmod(h, 2)
                    base = p * 64
                    es_cur = []
                    avps = (avp.tile([65, N], F32, tag="av", name="ps_av")
                            if h < H else None)
                    for jc in range(8):
                        if h < H:
                            stps = stp.tile([128, N], F32, tag="st", name="ps_st")
                            for ic in range(2):
                                nc.tensor.matmul(
                                    stps[:, NC2[ic]],
                                    kpack[hp][base:base + 64, jc * 128:(jc + 1) * 128],
                                    qpack[hp][base:base + 64, NC2[ic]],
                                    start=True, stop=True,
                                )
                            es = espool.tile([128, N], BF16, tag="es", name="es")
                            nc.scalar.activation(es[:], stps[:], EXP,
                                                 scale=float(HD) ** -0.5)
                            es_cur.append(es)
                            if h == 0:
                                # pipeline fill: head 0 has no AV work yet and
                                # QK is paced by exp, leaving ~1us PE holes
                                # that would re-throttle the clock gate. Pad
                                # with same-weight matmuls into the stps tile
                                # exp just consumed (WAR-ordered by Tile).
                                for _ in range(3):
                                    nc.tensor.matmul(
                                        stps[0:96, 0:512], wqes[0][:, :96],
                                        wqes[0][:, :512], start=True, stop=True)
                        if h > 0:
                            for ic in range(2):
                                nc.tensor.matmul(
                                    av_prev[:, NC2[ic]],
                                    vpack[jc][:, (h - 1) * 65:h * 65],
                                    es_prev[jc][:, NC2[ic]],
                                    start=(jc == 0), stop=(jc == 7),
                                )
                    if h > 0:
                        drain_head(h - 1, av_prev)
                    es_prev, av_prev = es_cur, avps

                # fill the attention->epilogue dependency gap (waiting on the
                # denominator DMAs + reciprocal) with warm same-weight matmuls
                # so the normalize/z-projection matmuls run at 2.4 GHz
                for i in range(30):
                    wu = stp.tile([96, 512], F32, tag="st", name="wu3")
                    nc.tensor.matmul(wu[:], wqes[0][:, :96], wqes[0][:, :512],
                                     start=True, stop=True)

            # ---- phase E: normalize (fp32) + output projections (bf16) ----
            with tc.tile_pool(name="on", bufs=1) as onpool, \
                 tc.tile_pool(name="z", bufs=4) as zpool, \
                 tc.tile_pool(name="ep", bufs=2, space="PSUM") as epp:
                otn = [onpool.tile([96, N], BF16, name=f"on{i}") for i in range(4)]
                oetn = {(r, k): onpool.tile([96, N], BF16, name=f"oen{r}{k}")
                        for r in range(2) for k in range(2)}
                rec = opool.tile([12, N], F32)
                nc.vector.reciprocal_approx_fast(rec[:], rin[:])
                bc1 = epp.tile([96, N], F32, tag="bc", name="ps_bc")
                for ic in range(2):
                    nc.tensor.matmul(bc1[:, NC2[ic]], eb1[:], rec[:, NC2[ic]],
                                     start=True, stop=True)
                for bi in range(4):
                    nc.vector.tensor_mul(otn[bi][:], o1d[:, bi * N:(bi + 1) * N],
                                         bc1[:])
                for k in range(2):
                    bce = epp.tile([96, N], F32, tag="bc", name="ps_bc")
                    for ic in range(2):
                        nc.tensor.matmul(bce[:, NC2[ic]],
                                         ebe[:, k * 96:(k + 1) * 96],
                                         rec[:, NC2[ic]], start=True, stop=True)
                    for r in range(2):
                        nc.vector.tensor_mul(oetn[r, k][:],
                                             oe[k][:, r * N:(r + 1) * N], bce[:])

                for zi in range(4):
                    zs = zpool.tile([96, N], F32, tag="z", name="zs")
                    for ic in range(2):
                        ps = epp.tile([96, 512], F32, tag="zps", name="ps_z")
                        nc.tensor.matmul(ps[:], wps[zi][:], otn[zi][:, NC2[ic]],
                                         start=True, stop=True)
                        if zi == 0:
                            nc.vector.tensor_scalar_add(zs[:, NC2[ic]], ps[:],
                                                        bpa1[:])
                        else:
                            nc.vector.tensor_copy(zs[:, NC2[ic]], ps[:])
                    nc.sync.dma_start(zd[zi][:], zs[:])
                for r in range(2):
                    for mc in range(2):
                        zs = zpool.tile([96, N], F32, tag="z", name="zs")
                        for ic in range(2):
                            ps = epp.tile([96, 512], F32, tag="zps", name="ps_z")
                            for k in range(2):
                                nc.tensor.matmul(
                                    ps[:], wpes[k][:, mc * 96:(mc + 1) * 96],
                                    oetn[r, k][:, NC2[ic]],
                                    start=(k == 0), stop=(k == 1),
                                )
                            nc.vector.tensor_copy(zs[:, NC2[ic]], ps[:])
                        nc.sync.dma_start(zed[r][mc * 96:(mc + 1) * 96, :], zs[:])

    nc.compile()
    return nc


def make_in_maps(inputs):
    from ml_dtypes import bfloat16

    b16 = lambda a: np.ascontiguousarray(np.asarray(a, dtype=np.float32)).astype(bfloat16)  # noqa: E731
    f32c = lambda a: np.ascontiguousarray(a, dtype=np.float32)  # noqa: E731
    ones = np.ones((1, N), np.float32)
    shared = {
        "wqa1t": b16(np.concatenate(
            [np.asarray(inputs["wq_A1"], np.float32).T,
             np.asarray(inputs["bq_A1"], np.float32)[None, :]], 0)),
        "wqa2t": b16(np.asarray(inputs["wq_A2"]).T),
        "wqb1t": b16(np.asarray(inputs["wq_B1"]).T),
        "wqb2t": b16(np.asarray(inputs["wq_B2"]).T),
        "wpa1t": b16(np.asarray(inputs["wp_A1"]).T),
        "wpa2t": b16(np.asarray(inputs["wp_A2"]).T),
        "wpb1t": b16(np.asarray(inputs["wp_B1"]).T),
        "wpb2t": b16(np.asarray(inputs["wp_B2"]).T),
        "eb1": (np.arange(96)[None, :] // 8 == np.arange(12)[:, None]).astype(np.float32),
        "ebe": (np.arange(192)[None, :] // 16 == np.arange(12)[:, None]).astype(np.float32),
        "bpa1": f32c(np.asarray(inputs["bp_A1"], np.float32)[:, None]),
    }
    for k in range(2):
        shared[f"wqet_{k}"] = b16(np.asarray(inputs["wq_E"]).T[k * 96:(k + 1) * 96])
        shared[f"wpet_{k}"] = b16(np.asarray(inputs["wp_E"]).T[k * 96:(k + 1) * 96])
    maps = []
    for b in range(B):
        m = dict(shared)
        m["xa1t"] = b16(np.concatenate(
            [np.asarray(inputs["x_A1"][b], np.float32).T, ones], 0))
        m["xa2t"] = b16(np.asarray(inputs["x_A2"][b]).T)
        m["xb1t"] = b16(np.asarray(inputs["x_B1"][b]).T)
        m["xb2t"] = b16(np.asarray(inputs["x_B2"][b]).T)
        for r in range(2):
            for k in range(2):
                m[f"x2t_{r}_{k}"] = b16(
                    np.asarray(inputs["x_2d"][b, :, r, k * 96:(k + 1) * 96]).T)
        maps.append(m)
    return maps


def assemble_outputs(results):
    z = [np.empty((B, N, 96), np.float32) for _ in range(4)]
    ze = np.empty((B, N, 2, 192), np.float32)
    for b in range(B):
        for i in range(4):
            z[i][b] = results[b][f"z{i + 1}t"].T
        for r in range(2):
            ze[b, :, r, :] = results[b][f"zet_{r}"].T
    return z[0], z[1], z[2], z[3], ze


_NC_CACHE = {}


def kernel(**inputs):
    if "nc" not in _NC_CACHE:
        _NC_CACHE["nc"] = build()
    nc = _NC_CACHE["nc"]
    res = run_bass_kernel_spmd(nc, make_in_maps(inputs), list(range(B)))
    return assemble_outputs(res.results)


# revision 24
# speedup vs baseline: 1.3319x; 1.0084x over previous
"""Trainium2 Bass kernel for nn_AttentionD8 (dense transformer attention, D8 irreps).

Sharding: data-parallel over batch B=8 -> 8 NeuronCores, one batch element per
core. No collectives. Each core runs the full per-batch attention block.

Device layout: channel-major ("transposed") everywhere; the host pre-transposes
inputs/weights with numpy and post-transposes outputs, so the device never
transposes anything:
  - qkv projections compute y.T = Wq @ x.T directly.
  - attention computes S.T (keys j on partitions, queries i on free axis);
    exp(S.T) is exactly the stationary operand of the A@V matmul, and an extra
    ones-column appended to V yields the softmax denominators for free.
  - normalization happens after attention: reciprocal of the denominators,
    PE-broadcast to (96 x N) tiles via 0/1 selector matmuls, one elementwise
    multiply per assembled output tile.
  - biases enter via a ones-row appended to x.T and a bias-row on the weight
    (bf16), except bp_A1 which is added in fp32 on the PSUM eviction.

Within-head channel orders are chosen so every partition-interleaving
SBUF->SBUF assembly is ONE dma whose flat element stream matches on both
sides:
  - q/k packs: 1d rows d*4+bi, 2d rows 32+e*2+r (a contraction-order
    permutation, consistent between q and k, so S is unchanged).
  - V-pack columns use the same order, so the attention-output rows stream
    straight into the merged o-tiles.

The PE clock-gate (HAM) has hysteresis: the attention QK/AV interleave is
~99% busy with micro-holes and never *transitions* the gate, so whatever
state it enters with persists. Dense same-weight warmup bursts are issued at
kernel start and right before attention to enter warm (2.4 GHz).

Matmul operands are bf16; accumulation is fp32 in PSUM; softmax statistics
and normalization are fp32.
"""

import os
import sys

import numpy as np

for _p in ("/opt/trn_rl_repo", os.path.expanduser("~/.axon_site/_ro/trn_rl_repo")):
    if _p not in sys.path and os.path.isdir(_p):
        sys.path.append(_p)

import concourse.bass as bass  # noqa: F401
import concourse.tile as tile
from concourse import bacc, mybir
from concourse.bass_utils import run_bass_kernel_spmd

F32 = mybir.dt.float32
BF16 = mybir.dt.bfloat16
EXP = mybir.ActivationFunctionType.Exp

B, N, H, C = 8, 1024, 12, 96
HD = 64
NC2 = (slice(0, 512), slice(512, 1024))


def build():
    nc = bacc.Bacc("TRN2", target_bir_lowering=False, debug=False, num_devices=8)

    def inp(name, shape, dt=BF16):
        return nc.dram_tensor(name, list(shape), dt, kind="ExternalInput").ap()

    def outp(name, shape):
        return nc.dram_tensor(name, list(shape), F32, kind="ExternalOutput").ap()

    # single big-row input/weight tensors: one DMA each instead of ~25
    # (the per-tensor issue rate on the sync sequencer was a 15us stall)
    xa1d = inp("xa1t", (97, N))
    bigxd = inp("bigx", (96, 7 * N))      # [xa2|xb1|xb2|x2_00|x2_01|x2_10|x2_11]
    wqa1d = inp("wqa1t", (97, 288))
    bigwd = inp("bigw", (96, 2784))       # [wqa2|wqb1|wqb2|wqet0|wqet1|wp1..4|wpet0|wpet1]
    eb1d = inp("eb1", (12, 96), F32)
    ebed = inp("ebe", (12, 192), F32)
    bpa1d = inp("bpa1", (96, 1), F32)

    zd = [outp("z1t", (96, N)), outp("z2t", (96, N)), outp("z3t", (96, N)),
          outp("z4t", (96, N))]
    zed = [outp("zet_0", (192, N)), outp("zet_1", (192, N))]

    with tile.TileContext(nc) as tc, \
         tc.tile_pool(name="w", bufs=1) as wpool, \
         tc.tile_pool(name="pack", bufs=1) as packpool:
        # ---- constant / weight loads (wqa1 first: it feeds the warmup) ----
        wqa1 = wpool.tile([97, 288], BF16)
        nc.sync.dma_start(wqa1[:], wqa1d[:])
        bigw = wpool.tile([96, 2784], BF16)
        nc.sync.dma_start(bigw[:], bigwd[:])
        wqs = [wqa1, bigw[:, 0:288], bigw[:, 288:576], bigw[:, 576:864]]
        wqes = [bigw[:, 864:1440], bigw[:, 1440:2016]]
        wps = [bigw[:, 2016 + i * 96: 2016 + (i + 1) * 96] for i in range(4)]
        wpes = [bigw[:, 2400 + k * 192: 2400 + (k + 1) * 192] for k in range(2)]
        eb1 = wpool.tile([12, 96], F32)
        nc.scalar.dma_start(eb1[:], eb1d[:])
        ebe = wpool.tile([12, 192], F32)
        nc.scalar.dma_start(ebe[:], ebed[:])
        bpa1 = wpool.tile([96, 1], F32)
        nc.scalar.dma_start(bpa1[:], bpa1d[:])

        qpack = [packpool.tile([128, N], BF16, name=f"qp{i}") for i in range(6)]
        kpack = [packpool.tile([128, N], BF16, name=f"kp{i}") for i in range(6)]
        vpack = [packpool.tile([128, 12 * 65], BF16, name=f"vp{i}") for i in range(8)]

        with tc.tile_pool(name="x", bufs=1) as xpool, \
             tc.tile_pool(name="y", bufs=1) as ypool, \
             tc.tile_pool(name="pj", bufs=2, space="PSUM") as pj:

            xa1 = xpool.tile([97, N], BF16)
            nc.gpsimd.dma_start(xa1[:], xa1d[:])
            bigx = xpool.tile([96, 7 * N], BF16)
            nc.gpsimd.dma_start(bigx[:], bigxd[:])
            xs = [xa1, bigx[:, 0:N], bigx[:, N:2 * N], bigx[:, 2 * N:3 * N]]
            x2s = {(r, k): bigx[:, (3 + r * 2 + k) * N:(4 + r * 2 + k) * N]
                   for r in range(2) for k in range(2)}

            # ---- PE warmup. The HAM busy-detector only fires on a dense
            # same-weight matmul stream (LDWEIGHTS micro-holes defeat it), so
            # real phases can never warm themselves; once warm, the state
            # persists as long as no idle gap exceeds ~3.4us. Burn matmuls on
            # the first-loaded weight tile, then chain onto bigw/bigx (same
            # stationary operand keeps the stream dense) so the burst cannot
            # finish before phase A's inputs have landed. ----
            for i in range(16):
                wu = pj.tile([96, 512], F32, tag="pj", name="wu")
                nc.tensor.matmul(wu[:, :288], wqa1[:, :96], wqa1[:, :288],
                                 start=True, stop=True)
            for i in range(4):
                wu = pj.tile([96, 512], F32, tag="pj", name="wu")
                nc.tensor.matmul(wu[:], wqa1[:96, :96], bigw[:, NC2[i % 2]],
                                 start=True, stop=True)
            for i in range(4):
                wu = pj.tile([96, 512], F32, tag="pj", name="wu")
                nc.tensor.matmul(wu[:], wqa1[:96, :96], bigx[:, NC2[i % 2]],
                                 start=True, stop=True)

            # y layout: 1d merged per qk -> (96 rows h*8+d) x (4 irreps x N);
            # 2d merged per (qk, head-half) -> (96 rows (h%6)*16+e) x (2 r x N)
            y1d = [ypool.tile([96, 4 * N], BF16, name=f"y1d{qk}") for qk in range(2)]
            y2d = {(qk, hh): ypool.tile([96, 2 * N], BF16, name=f"y2d{qk}{hh}")
                   for qk in range(2) for hh in range(2)}

            # ---- phase A: q/k projections ----
            for bi in range(4):
                for qk in range(2):
                    for ic in range(2):
                        ps = pj.tile([96, 512], F32, tag="pj", name="ps_pj")
                        nc.tensor.matmul(
                            ps[:],
                            wqs[bi][:, qk * 96:(qk + 1) * 96],
                            xs[bi][:, NC2[ic]],
                            start=True, stop=True,
                        )
                        nc.vector.tensor_copy(
                            y1d[qk][:, bi * N + ic * 512: bi * N + ic * 512 + 512],
                            ps[:])
            for r in range(2):
                for oc in range(4):  # oc = qk*2 + hh
                    qk, hh = divmod(oc, 2)
                    for ic in range(2):
                        ps = pj.tile([96, 512], F32, tag="pj", name="ps_pj")
                        for k in range(2):
                            nc.tensor.matmul(
                                ps[:],
                                wqes[k][:, oc * 96:(oc + 1) * 96],
                                x2s[r, k][:, NC2[ic]],
                                start=(k == 0), stop=(k == 1),
                            )
                        nc.vector.tensor_copy(
                            y2d[qk, hh][:, r * N + ic * 512: r * N + ic * 512 + 512],
                            ps[:])

            # ---- phase C: v projections, token-major, into packed v tiles ----
            # vpack column order per head: [1d: d*4+bi | 2d: 32+e*2+r | ones]
            for t8 in range(8):
                tok = slice(t8 * 128, (t8 + 1) * 128)
                v3 = vpack[t8].rearrange("p (h c) -> p h c", c=65)
                v1dv = v3[:, :, 0:32].rearrange("p h (d b) -> p h d b", b=4)
                v2dv = v3[:, :, 32:64].rearrange("p h (e r) -> p h e r", r=2)
                for bi in range(4):
                    ps = pj.tile([128, 96], F32, tag="pjv", name="ps_pjv")
                    nc.tensor.matmul(
                        ps[:], xs[bi][:, tok], wqs[bi][:, 192:288],
                        start=True, stop=True,
                    )
                    nc.vector.tensor_copy(
                        v1dv[:, :, :, bi],
                        ps.rearrange("p (h d) -> p h d", d=8),
                    )
                for r in range(2):
                    ps = pj.tile([128, 192], F32, tag="pjv", name="ps_pjv")
                    for k in range(2):
                        nc.tensor.matmul(
                            ps[:], x2s[r, k][:, tok], wqes[k][:, 384:576],
                            start=(k == 0), stop=(k == 1),
                        )
                    nc.vector.tensor_copy(
                        v2dv[:, :, :, r],
                        ps.rearrange("p (h e) -> p h e", e=16),
                    )
                nc.vector.memset(v3[:, :, 64:65], 1.0)

            # ---- phase B: assemble q/k packs. One dma per (head, qk, 1d/2d);
            # flat element streams match (dst is a plain 2D partition slice).
            # Issue alternates sync/gpsimd queues. ----
            for h in range(H):
                hp, p = divmod(h, 2)
                eng = [nc.sync, nc.gpsimd][h % 2]
                for qk, pack in ((0, qpack), (1, kpack)):
                    base = p * 64
                    eng.dma_start(
                        pack[hp][base: base + 32, :],
                        y1d[qk][h * 8:(h + 1) * 8, :].rearrange(
                            "d (b t) -> d b t", b=4),
                    )
                    eng.dma_start(
                        pack[hp][base + 32: base + 64, :],
                        y2d[qk, h // 6][(h % 6) * 16:(h % 6 + 1) * 16, :].rearrange(
                            "e (r t) -> e r t", r=2),
                    )

        # ---- phases D/E: attention, normalize, output projections ----
        with tc.tile_pool(name="o", bufs=1) as opool:
            # merged assembled outputs: o1d rows h*8+d, free (bi, t);
            # oe[kc] rows (h%6)*16+e, free (r, t)
            o1d = opool.tile([96, 4 * N], F32, name="o1d")
            oe = [opool.tile([96, 2 * N], F32, name=f"oe{kc}") for kc in range(2)]
            rin = opool.tile([12, N], F32)

            with tc.tile_pool(name="es", bufs=16) as espool, \
                 tc.tile_pool(name="avsp", bufs=3) as avspool, \
                 tc.tile_pool(name="st", bufs=2, space="PSUM") as stp, \
                 tc.tile_pool(name="av", bufs=2, space="PSUM") as avp:

                # bridge warmup: keep/restore HAM warm across the assembly gap
                for i in range(15):
                    wu = stp.tile([96, 512], F32, tag="st", name="wu2")
                    nc.tensor.matmul(wu[:], wqes[0][:, :96], wqes[0][:, :512],
                                     start=True, stop=True)

                def drain_head(h, avps):
                    avs = avspool.tile([65, N], F32, tag="avs", name="avs")
                    nc.vector.tensor_copy(avs[:], avps[:])
                    nc.gpsimd.dma_start(o1d[h * 8:(h + 1) * 8, :], avs[0:32, :])
                    nc.gpsimd.dma_start(
                        oe[h // 6][(h % 6) * 16:(h % 6 + 1) * 16, :], avs[32:64, :])
                    nc.gpsimd.dma_start(rin[h:h + 1, :], avs[64:65, :])

                es_prev, av_prev = None, None
                for h in range(H + 1):
                    hp, p = divmod(h, 2)
                    base = p * 64
                    es_cur = []
                    avps = (avp.tile([65, N], F32, tag="av", name="ps_av")
                            if h < H else None)
                    for jc in range(8):
                        if h < H:
                            stps = stp.tile([128, N], F32, tag="st", name="ps_st")
                            for ic in range(2):
                                nc.tensor.matmul(
                                    stps[:, NC2[ic]],
                                    kpack[hp][base:base + 64, jc * 128:(jc + 1) * 128],
                                    qpack[hp][base:base + 64, NC2[ic]],
                                    start=True, stop=True,
                                )
                            es = espool.tile([128, N], BF16, tag="es", name="es")
                            nc.scalar.activation(es[:], stps[:], EXP,
                                                 scale=float(HD) ** -0.5)
                            es_cur.append(es)
                            if h == 0:
                                # pipeline fill: head 0 has no AV work yet and
                                # QK is paced by exp, leaving ~1us PE holes
                                # that would re-throttle the clock gate. Pad
                                # with same-weight matmuls into the stps tile
                                # exp just consumed (WAR-ordered by Tile).
                                for _ in range(3):
                                    nc.tensor.matmul(
                                        stps[0:96, 0:512], wqes[0][:, :96],
                                        wqes[0][:, :512], start=True, stop=True)
                        if h > 0:
                            for ic in range(2):
                                nc.tensor.matmul(
                                    av_prev[:, NC2[ic]],
                                    vpack[jc][:, (h - 1) * 65:h * 65],
                                    es_prev[jc][:, NC2[ic]],
                                    start=(jc == 0), stop=(jc == 7),
                                )
                    if h > 0:
                        drain_head(h - 1, av_prev)
                    es_prev, av_prev = es_cur, avps

                # fill the attention->epilogue dependency gap (waiting on the
                # denominator DMAs + reciprocal) with warm same-weight matmuls
                # so the normalize/z-projection matmuls run at 2.4 GHz; the
                # rin-gated ones cannot run ahead of the denominator DMAs
                for i in range(12):
                    wu = stp.tile([96, 512], F32, tag="st", name="wu3")
                    nc.tensor.matmul(wu[:, :288], wqa1[:, :96], wqa1[:, :288],
                                     start=True, stop=True)
                for i in range(4):
                    wu = stp.tile([96, 512], F32, tag="st", name="wu3")
                    nc.tensor.matmul(wu[:, :N // 2], eb1[:], rin[:, NC2[i % 2]],
                                     start=True, stop=True)

            # ---- phase E: normalize (fp32) + output projections (bf16) ----
            with tc.tile_pool(name="on", bufs=1) as onpool, \
                 tc.tile_pool(name="z", bufs=4) as zpool, \
                 tc.tile_pool(name="ep", bufs=2, space="PSUM") as epp:
                otn = [onpool.tile([96, N], BF16, name=f"on{i}") for i in range(4)]
                oetn = {(r, k): onpool.tile([96, N], BF16, name=f"oen{r}{k}")
                        for r in range(2) for k in range(2)}
                rec = opool.tile([12, N], F32)
                nc.vector.reciprocal_approx_fast(rec[:], rin[:])
                bc1 = epp.tile([96, N], F32, tag="bc", name="ps_bc")
                for ic in range(2):
                    nc.tensor.matmul(bc1[:, NC2[ic]], eb1[:], rec[:, NC2[ic]],
                                     start=True, stop=True)
                for bi in range(4):
                    nc.vector.tensor_mul(otn[bi][:], o1d[:, bi * N:(bi + 1) * N],
                                         bc1[:])
                for k in range(2):
                    bce = epp.tile([96, N], F32, tag="bc", name="ps_bc")
                    for ic in range(2):
                        nc.tensor.matmul(bce[:, NC2[ic]],
                                         ebe[:, k * 96:(k + 1) * 96],
                                         rec[:, NC2[ic]], start=True, stop=True)
                    for r in range(2):
                        nc.vector.tensor_mul(oetn[r, k][:],
                                             oe[k][:, r * N:(r + 1) * N], bce[:])
                # keep the PE warm across the DVE normalize window so the
                # z-projections run at 2.4 GHz
                for i in range(16):
                    wu = epp.tile([96, 512], F32, tag="zps", name="wu4", bufs=4)
                    nc.tensor.matmul(wu[:, :288], wqa1[:, :96], wqa1[:, :288],
                                     start=True, stop=True)

                for zi in range(4):
                    zs = zpool.tile([96, N], F32, tag="z", name="zs")
                    for ic in range(2):
                        ps = epp.tile([96, 512], F32, tag="zps", name="ps_z",
                                      bufs=4)
                        nc.tensor.matmul(ps[:], wps[zi][:], otn[zi][:, NC2[ic]],
                                         start=True, stop=True)
                        if zi == 0:
                            nc.vector.tensor_scalar_add(zs[:, NC2[ic]], ps[:],
                                                        bpa1[:])
                        else:
                            nc.scalar.copy(zs[:, NC2[ic]], ps[:])
                    nc.sync.dma_start(zd[zi][:], zs[:])
                for r in range(2):
                    for mc in range(2):
                        zs = zpool.tile([96, N], F32, tag="z", name="zs")
                        for ic in range(2):
                            ps = epp.tile([96, 512], F32, tag="zps", name="ps_z",
                                          bufs=4)
                            for k in range(2):
                                nc.tensor.matmul(
                                    ps[:], wpes[k][:, mc * 96:(mc + 1) * 96],
                                    oetn[r, k][:, NC2[ic]],
                                    start=(k == 0), stop=(k == 1),
                                )
                            nc.scalar.copy(zs[:, NC2[ic]], ps[:])
                        nc.sync.dma_start(zed[r][mc * 96:(mc + 1) * 96, :], zs[:])

    nc.compile()
    return nc


def make_in_maps(inputs):
    from ml_dtypes import bfloat16

    b16 = lambda a: np.ascontiguousarray(np.asarray(a, dtype=np.float32)).astype(bfloat16)  # noqa: E731
    f32c = lambda a: np.ascontiguousarray(a, dtype=np.float32)  # noqa: E731
    f = lambda n: np.asarray(inputs[n], np.float32)  # noqa: E731
    ones = np.ones((1, N), np.float32)
    wqe_t = f("wq_E").T
    wpe_t = f("wp_E").T
    bigw = np.concatenate(
        [f("wq_A2").T, f("wq_B1").T, f("wq_B2").T,
         wqe_t[0:96], wqe_t[96:192],
         f("wp_A1").T, f("wp_A2").T, f("wp_B1").T, f("wp_B2").T,
         wpe_t[0:96], wpe_t[96:192]], axis=1)
    shared = {
        "wqa1t": b16(np.concatenate([f("wq_A1").T, f("bq_A1")[None, :]], 0)),
        "bigw": b16(bigw),
        "eb1": (np.arange(96)[None, :] // 8 == np.arange(12)[:, None]).astype(np.float32),
        "ebe": (np.arange(192)[None, :] // 16 == np.arange(12)[:, None]).astype(np.float32),
        "bpa1": f32c(f("bp_A1")[:, None]),
    }
    x2 = np.asarray(inputs["x_2d"], np.float32)
    maps = []
    for b in range(B):
        m = dict(shared)
        m["xa1t"] = b16(np.concatenate([f("x_A1")[b].T, ones], 0))
        m["bigx"] = b16(np.concatenate(
            [f("x_A2")[b].T, f("x_B1")[b].T, f("x_B2")[b].T,
             x2[b, :, 0, 0:96].T, x2[b, :, 0, 96:192].T,
             x2[b, :, 1, 0:96].T, x2[b, :, 1, 96:192].T], axis=1))
        maps.append(m)
    return maps


def assemble_outputs(results):
    z = [np.empty((B, N, 96), np.float32) for _ in range(4)]
    ze = np.empty((B, N, 2, 192), np.float32)
    for b in range(B):
        for i in range(4):
            z[i][b] = results[b][f"z{i + 1}t"].T
        for r in range(2):
            ze[b, :, r, :] = results[b][f"zet_{r}"].T
    return z[0], z[1], z[2], z[3], ze


_NC_CACHE = {}


def kernel(**inputs):
    if "nc" not in _NC_CACHE:
        _NC_CACHE["nc"] = build()
    nc = _NC_CACHE["nc"]
    res = run_bass_kernel_spmd(nc, make_in_maps(inputs), list(range(B)))
    return assemble_outputs(res.results)
